# revision 36
# baseline (speedup 1.0000x reference)
"""MoE (top-2 of 8 experts) Trainium2 kernel.

Strategy (expert-parallel over 8 NeuronCores):
  - Router runs on host (~0.1% of FLOPs); it defines the dispatch.
  - Each core e receives the tokens routed to expert e (gathered, transposed
    to [D, C], zero-padded to capacity C) plus expert e's weights, and runs
    the 3-layer MLP on-device in a transposed dataflow:
        h1T = relu(W1^T x^T + b1)   [H,  C]
        h2T = relu(W2^T h1T + b2)   [H2, C]
        yT  = W3^T h2T + b3         [O,  C]
  - Host combines per-expert outputs with the renormalized top-2 routing
    weights (scatter-add), matching the reference's dense-combine semantics.
  - Matmuls in bf16 with fp32 PSUM accumulation.
  - Capacity C = 2184 trims padding to the measured max expert load; any
    overflow beyond C is handled by extra (small) rounds, so correctness
    never depends on C.

Perf structure (evolved from a 262us baseline via NTFF trace analysis):
  - First token tile is 256 wide so the first matmul is gated by ~0.5MB of
    critical DMA instead of 1.25MB; w1's m0 tile rides the gpsimd SWDGE
    queue (a third queue whose ~2us fixed cost overlaps the HWDGE ramp),
    so real PE work starts at ~12us instead of ~15us.  8 dummy warm-up
    matmuls bridge the DMA-only window so the PE HAM clock-gate lifts
    (K=8/8) before the real stream begins.
  - x is shipped tile-major ([P, KD*tw] contiguous per tile) so each
    x-tile DMA is a simple 2D AP (a 3D strided AP cost ~5.5us of trigger
    time on the issuing engine).
  - Everything streams across both HWDGE queues in exact consume order
    (x0 k-halves lead both queues; w1 then w2 in single-m-tile groups
    alternating queues; w3 late; then x1..x4).  Measured: sync ~190GB/s
    from ~10us, scalar ~172GB/s from ~11.5us.
  - L3 (O=10) runs col-tiled: k-chunk pairs accumulate into 4 concurrent
    32-row PE column groups, then a 4-op DVE chain reduces the groups.
    The k=7 matmul (the only one that waits on the m7 relu evacuation)
    plus the final reduce op and the output DMA are DEFERRED into the
    next tile's first L1 m-group, so the PE never idles on the DVE at
    tile boundaries (was ~0.9us per boundary plus a 2.6us wait at the
    last tile).
  - h1/h2 live in per-chunk tiles so consumer matmuls wait only on the
    chunk they read; one 1-element DVE fence per tile (reading the
    previous tile's last inline DVE write) absorbs all older own-engine
    ticks so no instruction needs a second sync wait (the ISA carries a
    single sync wait per instruction and codegen rejects more).
  - Outputs leave per-tile via gpsimd SWDGE (HWDGE outputs would need a
    ring-throttle wait on top of the data wait - two sync waits).  A
    [106,tw] unreduced strip-ship to host was tried instead of the DVE
    chain: the chain costs ~2.3us DVE but the big strided SWDGE write
    crawled at ~25GB/s (7us drain tail), so the chain wins.
  - Run-to-run variance: the chip sometimes runs the whole kernel at
    2.0GHz (P0 power state) - +-20% on any single measurement.
"""

import re as _re

import numpy as np
import ml_dtypes

import bass_rust as _bass_rust
import concourse.bass as bass
import concourse.mybir as mybir
import concourse.tile as tile
from concourse.bass_utils import run_bass_kernel_spmd


def _split_drain_and_barrier(self, tick_clock, wait_clock):
    """Replacement for TileContext._drain_and_barrier.

    The stock version hangs every outstanding proc semaphore wait on one
    Drain instruction; the walrus in this environment rejects any
    instruction carrying more than one sync wait. Emit the same waits as
    individual sync-engine wait_ge instructions (one wait each) before a
    clean drain instead.
    """
    ticks = [
        int(v)
        for v in _re.findall(r"\d+", repr(tick_clock.global_clock))
    ]
    for proc, sem in sorted(self.sems.allocated().items()):
        if proc < len(ticks) and ticks[proc] > 0:
            self.nc.sync.wait_ge(sem, _bass_rust.tick_to_sem(ticks[proc], proc))
    self.nc.sync.drain()

    self.nc.all_engine_barrier()
    assert self.sems is not None
    popped = self.nc._tile_sem_poison_stack.pop()
    assert popped is self._sem_poison
    # Host-side bookkeeping of clear_and_free_semaphores WITHOUT emitting
    # the gpsimd dma_reset/sem_clear instructions or the second
    # all_engine_barrier (~1.2us of measured teardown).  The NEFF executes
    # once per load, so the dirty semaphore values are never re-observed;
    # all outputs are already fenced by the drain + first barrier above.
    sem_nums = [
        s.num if hasattr(s, "num") else s
        for s in self.sems.allocated().values()
    ]
    if sem_nums:
        self.nc._state.prepend_free_semaphores(sem_nums)
        for poison_set in self.nc._tile_sem_poison_stack:
            poison_set.update(sem_nums)


tile.TileContext._drain_and_barrier = _split_drain_and_barrier

B, D, H, E, O, TOP_K = 8192, 1024, 2048, 8, 10, 2
H2 = H // 2
NCORES = 8
P = 128

TWS = [256, 512, 512, 512, 392]   # token tile widths (<=512 = one PSUM bank)
C = sum(TWS)                      # per-expert token capacity (tokens, padded)
OVERFLOW_TWS = [512]              # small NEFF for the (never-seen) case of
                                  # an expert exceeding C tokens
KD = D // P       # 8   k-chunks for layer 1
MH = H // P       # 16  m-tiles for layer 1 / k-chunks for layer 2
MH2 = H2 // P     # 8   m-tiles for layer 2 / k-chunks for layer 3

BF16 = mybir.dt.bfloat16
F32 = mybir.dt.float32
_nbf16 = ml_dtypes.bfloat16


NW1 = KD * H          # w1 columns in the packed weight tile
NW2 = MH * H2         # w2 columns
NW3 = MH2 * O         # w3 columns
L3T = True            # col-tiled layer 3

# w1 DMA groups: (first m-tile, n m-tiles, queue). m0 goes out on the
# gpsimd SWDGE queue immediately after the preamble (a third queue for the
# critical first matmul); all other m-tiles are singles alternating the two
# HWDGE queues in consume order, so the early L1 m-loop never outruns the
# stream (SWDGE turned out to be too slow for more than the first tile).
# m0 on gpsimd; m1+m2 both on sync (scalar's queue starts ~1.7us later and
# was measured delivering its early tiles ~2us behind the L1 m-loop), then
# even m on sync / odd m on scalar.
W1_GROUPS = [(0, 1, "g"), (1, 1, "a"), (2, 1, "a")] + [
    (m, 1, "a" if m % 2 == 0 else "s") for m in range(3, 16)
]


def _build_nc(with_bias: bool, tws) -> bass.Bass:
    cap = sum(tws)
    nc = bass.Bass()
    # Host pre-packs everything into the on-chip layout:
    #  xt   [P, KD*C]   - x gathered/transposed, tile-major: tile t occupies
    #                     columns [KD*off_t, KD*(off_t+tw)), k-chunks of tw
    #                     columns each -> every tile DMA is a contiguous 2D AP.
    #  w1/w2 packed m-major: for fixed m-tile the k-chunk blocks are adjacent.
    #  w3 packed k-major (tiny).
    xt = nc.dram_tensor("xt", [P, KD * cap], BF16, kind="ExternalInput")
    w1d = nc.dram_tensor("w1p", [P, NW1], BF16, kind="ExternalInput")
    w2d = nc.dram_tensor("w2p", [P, NW2], BF16, kind="ExternalInput")
    w3d = nc.dram_tensor("w3p", [P, NW3], BF16, kind="ExternalInput")
    if with_bias:
        bias = nc.dram_tensor("bias", [1, H + H2 + O], BF16, kind="ExternalInput")
    out = nc.dram_tensor("out", [O, cap], F32, kind="ExternalOutput")

    relu_kw = dict(op0=mybir.AluOpType.max)
    first_small = len(tws) > 1

    tok_offs = []
    off = 0
    for tw in tws:
        tok_offs.append(off)
        off += tw

    with tile.TileContext(nc) as tc:
        with (
            tc.tile_pool(name="weights", bufs=1) as wpool,
            tc.tile_pool(name="xin", bufs=1) as xpool,
            tc.tile_pool(name="ps1", bufs=4, space="PSUM") as ps1pool,
            tc.tile_pool(name="ps2", bufs=3, space="PSUM") as ps2pool,
            tc.tile_pool(name="ps3", bufs=1, space="PSUM") as ps3pool,
            tc.tile_pool(name="acts", bufs=2) as apool,
        ):
            # ---- DMA schedule ----------------------------------------
            # Exact consume order, split across sync (a), scalar (s) HWDGE
            # queues and the gpsimd SWDGE queue (g).  Measured queue rates:
            # sync ~190GB/s from ~10us, scalar ~172GB/s from ~11.5us; the
            # SWDGE carries w1's m0 so the first matmul is gated by two
            # 256KB transfers on independent queues.
            xsb_tiles = []
            for t, tw in enumerate(tws):
                xsb = xpool.tile([P, KD * tw], BF16, tag=f"x{t}")
                xsb_tiles.append(xsb)

            def xcol(t, k):
                return KD * tok_offs[t] + k * tws[t]

            # tile0's x in k-halves: first half leads the sync queue (gates
            # the very first matmul), second half leads the scalar queue.
            tw0 = tws[0]
            half = KD // 2
            nc.sync.dma_start(
                xsb_tiles[0][:, : half * tw0], xt[:, xcol(0, 0):xcol(0, half)])
            nc.scalar.dma_start(
                xsb_tiles[0][:, half * tw0:], xt[:, xcol(0, half):xcol(0, KD)])

            engs = {"a": nc.sync, "s": nc.scalar, "g": nc.gpsimd}
            w1g_tiles = []
            w1_group_of = {}
            for g, (m0_, nm, q) in enumerate(W1_GROUPS):
                w1g = wpool.tile([P, nm * KD * P], BF16, name=f"w1g{g}")
                engs[q].dma_start(
                    w1g, w1d[:, m0_ * KD * P:(m0_ + nm) * KD * P])
                w1g_tiles.append(w1g)
                for mm in range(m0_, m0_ + nm):
                    w1_group_of[mm] = (g, mm - m0_)

            # w3 late on scalar (tiny; first needed by tile0's L3 at ~35us).
            w3sb = wpool.tile([P, NW3], BF16)
            nc.scalar.dma_start(w3sb, w3d[:, :])

            # w2 in 8 single-m-tile groups, alternating queues in consume
            # order so tile0's L2 m-loop never outruns the stream.
            w2g_tiles = []
            for m in range(MH2):
                w2g = wpool.tile([P, MH * P], BF16, name=f"w2g{m}")
                eng = nc.sync if m % 2 == 0 else nc.scalar
                eng.dma_start(w2g, w2d[:, m * MH * P:(m + 1) * MH * P])
                w2g_tiles.append(w2g)

            # Remaining x tiles, alternating queues in consume order.
            for t in range(1, len(tws)):
                eng = nc.sync if t % 2 == 1 else nc.scalar
                eng.dma_start(
                    xsb_tiles[t], xt[:, xcol(t, 0):xcol(t, KD)])

            def w1s(k, m):
                g, mm_ = w1_group_of[m]
                off = (mm_ * KD + k) * P
                return w1g_tiles[g][:, off:off + P]

            def w2s(k, m):
                return w2g_tiles[m][:, k * P:(k + 1) * P]

            def w3s(k):
                off = k * O
                return w3sb[:, off:off + O]

            if with_bias:
                # Bias folded into each accumulation group as one extra K=1
                # matmul against a ones row: psum[m, n] += b[m] * 1.
                bsb = wpool.tile([1, H + H2 + O], BF16)
                nc.sync.dma_start(bsb, bias[:, :])
                ones = wpool.tile([1, max(tws)], BF16)
                nc.vector.memset(ones, 1.0)

            def bias_mm(ps, lo, hi, tw, **kw):
                if with_bias:
                    nc.tensor.matmul(
                        ps, bsb[:, lo:hi], ones[:, :tw], start=False, stop=True,
                        **kw,
                    )

            # Scratch row: the warm-up dummies read it, and the per-tile
            # 1-element fence copy (see emit_l1) writes its first column.
            fence = wpool.tile([1, 4], BF16)
            nc.vector.memset(fence, 0.0)

            # HAM warm-up: full-array dummy matmuls on zeroed scratch
            # bridge the DMA-only window (~8.3us to ~12us) so the PE
            # clock-gate lifts before the real matmul stream begins.
            if first_small:
                warm_w = wpool.tile([P, P], BF16, name="warm_w")
                warm_in = wpool.tile([P, 512], BF16, name="warm_in")
                nc.vector.memset(warm_w, 0.0)
                nc.vector.memset(warm_in, 0.0)
                warm_ps = ps1pool.tile([P, 512], F32, tag="ps1", name="warm")
                for _ in range(11):
                    nc.tensor.matmul(
                        warm_ps, warm_w, warm_in,
                        start=True, stop=True, skip_group_check=True,
                    )

            h1_of = {}
            fence_of = {}
            pending = [None]

            def emit_l1(t):
                tw = tws[t]
                xsb = xsb_tiles[t]
                # One fence per tile: a 1-element DVE read of the previous
                # tile's last inline DVE write (its L3 reduce temp) absorbs
                # every older own-engine WAW/WAR tick in one wait, so the
                # per-chunk activation tiles below never need a second sync
                # wait.
                if t >= 1:
                    nc.vector.tensor_copy(
                        fence[:, 0:1], fence_of[t - 1][0:1, 0:1])
                # Per-chunk h1 tiles: precise region deps, so L2's first
                # matmuls never wait on the last h1 evacuation.
                h1sb = [apool.tile([P, tw], BF16, tag=f"h1_{m}", name=f"h1_{m}") for m in range(MH)]
                h1_of[t] = h1sb
                for m in range(MH):
                    ps = ps1pool.tile([P, 512], F32, tag="ps1", name="ps1t")[:, :tw]
                    for k in range(KD):
                        nc.tensor.matmul(
                            ps,
                            w1s(k, m),
                            xsb[:, k * tw:(k + 1) * tw],
                            start=(k == 0),
                            stop=(k == KD - 1) and not with_bias,
                        )
                    bias_mm(ps, m * P, (m + 1) * P, tw)
                    nc.vector.tensor_scalar(
                        h1sb[m], ps, 0.0, None, **relu_kw
                    )
                    if m == 0 and pending[0] is not None:
                        # The previous tile's deferred L3 group 3 + reduce +
                        # output: its h2[7]-evac dependency completed during
                        # this tile's m0 matmuls, so the PE never stalls on
                        # the DVE at the tile boundary.
                        pending[0]()
                        pending[0] = None

            def emit_l23(t):
                tw = tws[t]
                tok = slice(tok_offs[t], tok_offs[t] + tw)
                last = t == len(tws) - 1
                h1sb = h1_of.pop(t)
                h2sb = [apool.tile([P, tw], BF16, tag=f"h2_{m}", name=f"h2_{m}") for m in range(MH2)]
                # Col-tiled L3: k-chunk pairs accumulate into 4 independent
                # 32-row PE column groups, then a DVE chain reduces the
                # groups (one PSUM operand per op; per-tile temps so the
                # first chain op never needs an own-engine wait on top of
                # the PE wait - the ISA wait slot fits one).
                # (A wide [106,tw] strip-ship to host was tried instead of
                # the chain: the 4-op chain costs ~2.3us DVE but the big
                # strided SWDGE write crawled at ~25GB/s - 7us drain tail.)
                ps3 = ps3pool.tile([P, 512], F32, tag="ps3", name="ps3t")
                osb = wpool.tile([O, tw], F32, name=f"osb{t}")
                t0 = wpool.tile([O, 512], F32, name=f"l3tmp0_{t}")[:, :tw]
                t1 = wpool.tile([O, 512], F32, name=f"l3tmp1_{t}")[:, :tw]
                fence_of[t] = t0

                def l3_mm(k, stop):
                    g = k // 2
                    nc.tensor.matmul(
                        ps3[32 * g:32 * g + O, :tw],
                        w3s(k),
                        h2sb[k],
                        start=(k % 2 == 0),
                        stop=stop and not (with_bias and g == 0),
                        tile_position=(0, 32 * g),
                        skip_group_check=True,
                    )

                def chain_op(g):
                    if g == 0:
                        bias_mm(ps3[0:O, :tw], H + H2, H + H2 + O, tw,
                                tile_position=(0, 0), skip_group_check=True)
                        nc.vector.tensor_copy(t0, ps3[0:O, :tw])
                    elif g == 1:
                        nc.vector.scalar_tensor_tensor(
                            t1, ps3[32:32 + O, :tw], 1.0, t0,
                            op0=mybir.AluOpType.mult, op1=mybir.AluOpType.add,
                        )
                    elif g == 2:
                        nc.vector.scalar_tensor_tensor(
                            t0, ps3[64:64 + O, :tw], 1.0, t1,
                            op0=mybir.AluOpType.mult, op1=mybir.AluOpType.add,
                        )
                    else:
                        nc.vector.scalar_tensor_tensor(
                            osb, ps3[96:96 + O, :tw], 1.0, t0,
                            op0=mybir.AluOpType.mult, op1=mybir.AluOpType.add,
                        )
                        # Per-tile SWDGE (gpsimd-issued) output transfer:
                        # overlaps with later compute and keeps the HWDGE
                        # queues' trigger streams single-wait (HWDGE outputs
                        # would need a ring-throttle wait on top of the data
                        # wait - two sync waits, which codegen rejects).
                        nc.gpsimd.dma_start(out[:, tok], osb)

                for m in range(MH2):
                    ps = ps2pool.tile([P, 512], F32, tag="ps2", name="ps2t")[:, :tw]
                    for k in range(MH):
                        nc.tensor.matmul(
                            ps,
                            w2s(k, m),
                            h1sb[k],
                            start=(k == 0),
                            stop=(k == MH - 1) and not with_bias,
                        )
                    bias_mm(ps, H + m * P, H + (m + 1) * P, tw)
                    nc.vector.tensor_scalar(
                        h2sb[m], ps, 0.0, None, **relu_kw
                    )
                    if last and m == 6:
                        # LAST tile: L3 groups 0-2 (still concurrent across
                        # column groups) + their chain ops go here, after
                        # the m6 evac - the chain then runs on the DVE
                        # UNDER m7's L2 matmuls, so only evac(m7) -> k6/k7
                        # -> one final reduce op -> output remain after the
                        # last L2 matmul (was: the whole 4-op chain, ~2.3us
                        # of serial DVE after the stream end).
                        for k in range(6):
                            l3_mm(k, stop=(k in (1, 3, 5)))
                        chain_op(0)
                        chain_op(1)
                        chain_op(2)
                if last:
                    l3_mm(6, stop=False)
                    l3_mm(7, stop=True)
                    # 1-element fence read of t0: its write (chain_op(2)) is
                    # non-adjacent on the DVE (the m7 evac sits in between),
                    # so without this the final reduce op would need an
                    # explicit own-engine wait ON TOP of its PE wait - two
                    # sync waits, which codegen rejects.  The fence carries
                    # the DVE wait alone; the final op then only waits PE.
                    nc.vector.tensor_copy(fence[:, 0:1], t0[0:1, 0:1])
                    chain_op(3)
                    return
                # Other tiles: 7 of the 8 L3 matmuls run concurrently (4
                # column groups) at the tile end; the k=7 matmul - the only
                # one that must wait on the m7 evacuation - plus the final
                # reduce + output DMA are deferred into the NEXT tile's
                # first L1 m-group, so the PE never idles at the boundary.
                for k in range(7):
                    l3_mm(k, stop=(k in (1, 3, 5)))
                chain_op(0)
                chain_op(1)
                chain_op(2)

                def deferred():
                    l3_mm(7, stop=True)
                    chain_op(3)
                pending[0] = deferred

            for t in range(len(tws)):
                emit_l1(t)
                emit_l23(t)
            assert pending[0] is None
    return nc


_NC_CACHE: dict = {}


def _get_nc(with_bias: bool, tws) -> bass.Bass:
    key = (with_bias, tuple(tws))
    if key not in _NC_CACHE:
        _NC_CACHE[key] = _build_nc(with_bias, tws)
    return _NC_CACHE[key]


def _route(x, Wr, br):
    """Host router: softmax over logits, top-2, renormalized weights."""
    logits = x.astype(np.float32) @ Wr.astype(np.float32) + br.astype(np.float32)
    m = logits.max(axis=-1, keepdims=True)
    p = np.exp(logits - m)
    p /= p.sum(axis=-1, keepdims=True)
    top_i = np.argsort(-p, axis=-1, kind="stable")[:, :TOP_K]
    top_p = np.take_along_axis(p, top_i, axis=-1)
    top_p = top_p / top_p.sum(axis=-1, keepdims=True)
    return top_i.astype(np.int64), top_p.astype(np.float32)


def _pack_weights(W1, b1, W2, b2, W3, b3, with_bias):
    w_maps = []
    for e in range(NCORES):
        m = {
            # w1 m-major: [p, m, k, c] so the first m-groups lead the DMA
            "w1p": np.ascontiguousarray(
                W1[e].reshape(KD, P, MH, P).transpose(1, 2, 0, 3).reshape(P, NW1)
            ).astype(_nbf16),
            # w2 m-major too (delivered in single-m groups)
            "w2p": np.ascontiguousarray(
                W2[e].reshape(MH, P, MH2, P).transpose(1, 2, 0, 3).reshape(P, NW2)
            ).astype(_nbf16),
            "w3p": np.ascontiguousarray(
                W3[e].reshape(MH2, P, O).transpose(1, 0, 2).reshape(P, NW3)
            ).astype(_nbf16),
        }
        if with_bias:
            m["bias"] = np.concatenate(
                [b1[e], b2[e], b3[e]]
            ).reshape(1, H + H2 + O).astype(_nbf16)
        w_maps.append(m)
    return w_maps


def _pack_x(x, tok, tws):
    """Gather tokens, transpose, and lay out tile-major [P, KD*cap] bf16."""
    cap = sum(tws)
    xt = np.zeros((P, KD * cap), _nbf16)
    n = len(tok)
    if n:
        # [n, D] -> [D, n] -> k-chunks [KD, P, n] -> [P, KD, n]
        xg = x[tok].astype(_nbf16).T.reshape(KD, P, n).transpose(1, 0, 2)
        off = 0
        for tw in tws:
            lo = off
            hi = min(off + tw, n)
            if lo >= n:
                break
            blk = np.zeros((P, KD, tw), _nbf16)
            blk[:, :, : hi - lo] = xg[:, :, lo:hi]
            xt[:, KD * off:KD * (off + tw)] = blk.reshape(P, KD * tw)
            off += tw
    return np.ascontiguousarray(xt)


def _run_rounds(x, top_i, top_p, W1, b1, W2, b2, W3, b3, trace=False):
    """Dispatch tokens to expert-owning cores, run the NEFF, combine."""
    with_bias = bool(np.any(b1) or np.any(b2) or np.any(b3))
    w_maps = _pack_weights(W1, b1, W2, b2, W3, b3, with_bias)

    # (token, slot) pairs per expert.
    tok_by_e = []
    wt_by_e = []
    for e in range(NCORES):
        tok, slot = np.nonzero(top_i == e)
        tok_by_e.append(tok)
        wt_by_e.append(top_p[tok, slot])

    out = np.zeros((B, O), np.float32)
    offset = [0] * NCORES
    last_result = None
    first_round = True
    while True:
        active = [e for e in range(NCORES) if offset[e] < len(tok_by_e[e])]
        if not active and last_result is not None:
            break
        # Round 1 uses the full-capacity NEFF. In the (never-observed) case
        # that an expert got more than C tokens, the leftovers run through a
        # small single-tile NEFF instead of paying for a full rerun.
        tws = TWS if first_round else OVERFLOW_TWS
        cap = sum(tws)
        nc = _get_nc(with_bias, tws)
        first_round = False
        in_maps = []
        chunks = []
        for e in range(NCORES):
            tok = tok_by_e[e][offset[e]:offset[e] + cap]
            chunks.append(tok)
            in_maps.append({"xt": _pack_x(x, tok, tws), **w_maps[e]})
        res = run_bass_kernel_spmd(
            nc, in_maps, core_ids=list(range(NCORES)), trace=trace
        )
        last_result = res
        for e in range(NCORES):
            tok = chunks[e]
            if len(tok) == 0:
                continue
            y = res.results[e]["out"][:, :len(tok)].T  # [n_e, O]
            w = wt_by_e[e][offset[e]:offset[e] + len(tok)]
            np.add.at(out, tok, w[:, None] * y)
            offset[e] += len(tok)
    return out, last_result


def kernel(x, Wr, br, W1, b1, W2, b2, W3, b3):
    x = np.asarray(x, np.float32)
    top_i, top_p = _route(x, np.asarray(Wr), np.asarray(br))
    out, _ = _run_rounds(
        x, top_i, top_p,
        np.asarray(W1), np.asarray(b1), np.asarray(W2), np.asarray(b2),
        np.asarray(W3), np.asarray(b3),
    )
    return out


def run_traced(x, Wr, br, W1, b1, W2, b2, W3, b3):
    """Like kernel() but returns (out, BassKernelResults) with profile info."""
    x = np.asarray(x, np.float32)
    top_i, top_p = _route(x, np.asarray(Wr), np.asarray(br))
    return _run_rounds(
        x, top_i, top_p,
        np.asarray(W1), np.asarray(b1), np.asarray(W2), np.asarray(b2),
        np.asarray(W3), np.asarray(b3),
        trace=True,
    )
